# revision 26
# baseline (speedup 1.0000x reference)
"""Depthwise cross-correlation (DepthwiseRPN) on 8 TRN2 NeuronCores.

Reference op:
  z_f: [B=128, C=256, 7, 7]   per-(b,c) kernels
  x_f: [B=128, C=256, 31, 31] search windows
  out: [B=128, C=256, 25, 25] valid cross-correlation per (b,c)

Sharding: pure data-parallel over B (16 batches per core).

Depthwise conv has no operand shared across a matmul grid, so TensorE
can only do ~128 useful MACs/cycle (diagonal weights; rhs-ingest
bound).  To beat the PE-only floor (~420 us/core) the per-core work is
split across three parallel pipelines by channel group (128 ch each):

  - PE groups (22): per-tap diagonal matmul, 49 taps accumulate in
    PSUM: psum[c,:] += diag(z[:,u,v]) @ x[:, shifted-window AP].
  - DVE groups (4): fused MAC via the AFFINE_THEN_ADD custom DVE op:
    acc = x_win*z_tap + acc  (bf16 reads, fp32 accumulator).
  - ACT+DVE groups (6): ScalarE mult (activation Copy with per-partition
    scale) into slots of [128,8,625] bf16 tiles; DVE folds each 8-slot
    tile with a batched binary tree (2500/1250/625-wide adds, all 2x
    mode), then combines the chain sums.

Scheduling (the v1->v2 delta, worth ~20 us):
  - PSUM pools are 4 deep and each round's evacuation is emitted BEFORE
    that round's matmuls, so the evac sits AHEAD of the round's AG mults
    in ACT's queue.  v1 accumulated ~0.6 us/round of ACT backlog which
    stalled PE 11 us near the end (PSUM starvation).
  - Group 0's x/zd DMAs are split fine (rows 0:7/7:14/14:20/20:31 and
    zd taps 0:1/1:8/8:49) and its matmuls run pA-before-pB so tap-0
    only waits on x rows 0:20 + zd tap 0 (~5 us startup vs 11.5).
  - Non-PE groups return bf16 (host upcasts); final DVE op writes the
    bf16 out tile directly.
Measured: ~334 us (v1) -> target ~310 us, max rel err ~6e-3.
"""

import numpy as np
import ml_dtypes

import concourse.bass as bass
import concourse.mybir as mybir
import concourse.tile as tile
from concourse import bacc
from concourse.bass_utils import run_bass_kernel_spmd

B, C = 128, 256
HX, WX = 31, 31
HZ, WZ = 7, 7
HO, WO = HX - HZ + 1, WX - WZ + 1  # 25, 25
NCORES = 8
BPC = B // NCORES         # batches per core = 16
Q = BPC * C               # (b,c) channels per core = 4096
G = Q // 128              # groups of 128 channels = 32
NX = HX * WX              # 961
NO = HO * WO              # 625
NT = HZ * WZ              # 49 taps
ROWS_A = 20               # psum chunk A rows (20*25=500 <= 512)
ROWS_B = HO - ROWS_A      # 5 rows (125)

# channel-group split across engines
G_PE = 21                 # TensorE diag-matmul groups
G_DVE = 4                 # DVE fused-MAC (AFFINE_THEN_ADD) groups
G_AG = G - G_PE - G_DVE   # 7: ACT-mult + DVE tree-add groups (last one split
                          # between ACT taps and DVE-affine taps)
SPLIT_AG_TAPS = 32        # last group: taps [0,32) on ACT path, rest on DVE

PSUM_BUFS = 4             # psA 4*2000B + psB 4*500B = 10KB <= 16KB
EVAC_DEFER = 3            # evac(g-EVAC_DEFER) emitted at round-g start

BF16 = ml_dtypes.bfloat16

_built = {}


def _ensure_ntff_hook():
    """Install the axon NTFF profiling hook if the container's antenv stub
    lacks it (needed only for trace=True local profiling runs)."""
    import contextlib
    import ctypes
    import sys
    import types

    try:
        from antenv.axon_hooks import get_axon_ntff_profile_hook  # noqa: F401

        return True
    except ImportError:
        pass
    so_path = "/opt/axon/libaxon_pjrt.so"
    try:
        lib = ctypes.CDLL(so_path)
    except OSError:
        return False
    if not hasattr(lib, "axon_start_nrt_profile"):
        return False
    lib.axon_start_nrt_profile.argtypes = [
        ctypes.POINTER(ctypes.c_int64),
        ctypes.c_size_t,
    ]
    lib.axon_start_nrt_profile.restype = ctypes.c_int64
    lib.axon_stop_nrt_profile.argtypes = [ctypes.c_char_p]
    lib.axon_stop_nrt_profile.restype = ctypes.c_int64

    @contextlib.contextmanager
    def _hook(output_dir, device_ids):
        import jax

        jax.devices()
        if device_ids:
            ids = (ctypes.c_int64 * len(device_ids))(*device_ids)
            rc = lib.axon_start_nrt_profile(ids, len(device_ids))
        else:
            rc = lib.axon_start_nrt_profile(None, 0)
        if rc != 0:
            raise RuntimeError(f"axon_start_nrt_profile rc={rc}")
        try:
            yield
        finally:
            n = lib.axon_stop_nrt_profile(str(output_dir).encode())
            print(f"profile: {n} file(s) written to {output_dir}", file=sys.stderr)

    state = {"hook": _hook}
    mod = types.ModuleType("antenv.axon_hooks")
    mod.get_axon_ntff_profile_hook = lambda: state["hook"]
    mod.set_axon_ntff_profile_hook = lambda h: state.update(hook=h)
    import antenv

    sys.modules["antenv.axon_hooks"] = mod
    antenv.axon_hooks = mod
    return True


def _emit_g0_gating_dmas(nc, pools, x_d, zd_d):
    """Group 0's tap-0..19 inputs only — the minimum to get PE rolling.
    The matmul dep tracker waits on whole tiles, so the zd load is split
    into three tiles sized to land just before their taps are reached."""
    xp, zp = pools["xp"], pools["zp"]
    x_sb = xp.tile([128, HX, WX], mybir.dt.bfloat16, name="xpe0", tag="xpe")
    zd_a = zp.tile([128, 8, 128], mybir.dt.bfloat16, name="zda", tag="zda", bufs=1)
    zd_b = zp.tile([128, 12, 128], mybir.dt.bfloat16, name="zdb", tag="zdb", bufs=1)
    x_src = x_d[0].rearrange("p (h w) -> p h w", h=HX)
    nc.sync.dma_start(out=zd_a, in_=zd_d[0][:, 0:8])
    nc.sync.dma_start(out=x_sb[:, 0:26], in_=x_src[:, 0:26])
    nc.sync.dma_start(out=zd_b, in_=zd_d[0][:, 8:20])
    return x_sb, x_src, zd_a, zd_b


def _emit_g0_rest(nc, pools, zd_d, x_sb, x_src, zd_a, zd_b):
    """The remaining g0 loads + the pA-then-pB matmul sweeps."""
    zp, psA, psB = pools["zp"], pools["psA"], pools["psB"]
    zd_c = zp.tile([128, NT - 20, 128], mybir.dt.bfloat16, name="zdc", tag="zdc", bufs=1)
    # dispatch from the (idle) ACT queue: DGE generation runs in parallel
    # with the sync queue's serialized dispatch stream
    nc.scalar.dma_start(out=zd_c, in_=zd_d[0][:, 20:])
    nc.scalar.dma_start(out=x_sb[:, 26:HX], in_=x_src[:, 26:HX])

    def lhs(t):
        if t < 8:
            return zd_a[:, t, :]
        if t < 20:
            return zd_b[:, t - 8, :]
        return zd_c[:, t - 20, :]

    pA = psA.tile([128, ROWS_A * WO], mybir.dt.float32, name="pA0", tag="pA")
    pB = psB.tile([128, ROWS_B * WO], mybir.dt.float32, name="pB0", tag="pB")
    for t in range(NT):
        u, v = divmod(t, WZ)
        nc.tensor.matmul(
            pA[:, :], lhs(t), x_sb[:, u : u + ROWS_A, v : v + WO],
            start=(t == 0), stop=(t == NT - 1),
        )
    for t in range(NT):
        u, v = divmod(t, WZ)
        nc.tensor.matmul(
            pB[:, :], lhs(t),
            x_sb[:, ROWS_A + u : ROWS_A + u + ROWS_B, v : v + WO],
            start=(t == 0), stop=(t == NT - 1),
        )
    return pA, pB


def _emit_pe_matmuls(nc, pools, x_d, zd_d, g):
    """Emit one PE group's DMAs + 49 tap matmul pairs; return psum tiles
    for a deferred evacuation."""
    xp, zp, psA, psB = pools["xp"], pools["zp"], pools["psA"], pools["psB"]
    x_sb = xp.tile([128, HX, WX], mybir.dt.bfloat16, name=f"xpe{g}", tag="xpe")
    zd_sb = zp.tile([128, NT, 128], mybir.dt.bfloat16, name=f"zd{g}", tag="zd")
    x_src = x_d[g].rearrange("p (h w) -> p h w", h=HX)
    nc.sync.dma_start(out=x_sb, in_=x_src)
    nc.sync.dma_start(out=zd_sb, in_=zd_d[g])

    pA = psA.tile([128, ROWS_A * WO], mybir.dt.float32, name=f"pA{g}", tag="pA")
    pB = psB.tile([128, ROWS_B * WO], mybir.dt.float32, name=f"pB{g}", tag="pB")
    if g == G_PE - 1:
        # last group: pA finishes ~6us before pB so its evac overlaps
        # the pB sweep, trimming the tail
        for t in range(NT):
            u, v = divmod(t, WZ)
            nc.tensor.matmul(
                pA[:, :], zd_sb[:, t, :], x_sb[:, u : u + ROWS_A, v : v + WO],
                start=(t == 0), stop=(t == NT - 1),
            )
        for t in range(NT):
            u, v = divmod(t, WZ)
            nc.tensor.matmul(
                pB[:, :], zd_sb[:, t, :],
                x_sb[:, ROWS_A + u : ROWS_A + u + ROWS_B, v : v + WO],
                start=(t == 0), stop=(t == NT - 1),
            )
    else:
        for t in range(NT):
            u, v = divmod(t, WZ)
            lhsT = zd_sb[:, t, :]
            nc.tensor.matmul(
                pA[:, :], lhsT, x_sb[:, u : u + ROWS_A, v : v + WO],
                start=(t == 0), stop=(t == NT - 1),
            )
            nc.tensor.matmul(
                pB[:, :], lhsT, x_sb[:, ROWS_A + u : ROWS_A + u + ROWS_B, v : v + WO],
                start=(t == 0), stop=(t == NT - 1),
            )
    return pA, pB


def _emit_pe_evac(nc, pools, out_d, g, pA, pB):
    op = pools["op"]
    out_sb = op.tile([128, NO], mybir.dt.float32, name=f"ope{g}", tag="ope")
    # ScalarE is closest to PSUM; keep DVE free for its MAC pipeline
    nc.scalar.copy(out=out_sb[:, : ROWS_A * WO], in_=pA[:, :])
    nc.scalar.copy(out=out_sb[:, ROWS_A * WO :], in_=pB[:, :])
    nc.sync.dma_start(out=out_d[g], in_=out_sb)


def _gen_dve_groups(nc, pools, xv_d, xo_d, zf_d, outv_d, groups, taps_per_yield=4):
    """Generator: DVE fused-MAC pipeline over `groups`, yielding every few
    taps so the driver can interleave c-group adds into DVE's stream."""
    xp, zp, op, ob = pools["xv"], pools["zf"], pools["ov"], pools["ovb"]
    for i, g in enumerate(groups):
        # stride-32 rows + an odd-shifted copy keep every window read
        # 4B-aligned (bf16 reads at odd element offsets run ~2x slower).
        # Both layouts are prepared host-side so the DMA is 128 contiguous
        # descriptors (a strided on-the-fly copy is ~4000 tiny descriptors
        # and serializes the sync queue for multiple us per load).
        x_e = xp.tile([128, HX, 32], mybir.dt.bfloat16, name=f"xdve{g}", tag="xdve")
        x_o = xp.tile([128, HX, 32], mybir.dt.bfloat16, name=f"xdvo{g}", tag="xdvo")
        zf_sb = zp.tile([128, NT], mybir.dt.float32, name=f"zfv{g}", tag="zfv")
        nc.sync.dma_start(out=x_e, in_=xv_d[i].rearrange("p (h w) -> p h w", h=HX))
        nc.sync.dma_start(out=x_o, in_=xo_d[i].rearrange("p (h w) -> p h w", h=HX))
        nc.sync.dma_start(out=zf_sb, in_=zf_d[g - G_PE])

        acc = op.tile([128, HO, WO], mybir.dt.float32, name=f"accv{g}", tag="accv")
        outt = ob.tile([128, HO, WO], mybir.dt.bfloat16, name=f"ovb{g}", tag="ovb")
        for t in range(NT):
            u, v = divmod(t, WZ)
            if v % 2 == 0:
                win = x_e[:, u : u + HO, v : v + WO]
            else:
                win = x_o[:, u : u + HO, v - 1 : v - 1 + WO]
            if t == 0:
                # seed on DVE (2x_2p tensor-scalar); ACT stays free for
                # evacs + AG mults
                nc.vector.tensor_scalar_mul(acc, win, zf_sb[:, 0:1])
            elif t < NT - 1:
                nc.vector.affine_then_add(acc, win, acc, zf_sb[:, t : t + 1], 0.0)
            else:
                # last tap writes the bf16 out tile directly
                nc.vector.affine_then_add(outt, win, acc, zf_sb[:, t : t + 1], 0.0)
            if (t + 1) % taps_per_yield == 0:
                yield
        nc.sync.dma_start(out=outv_d[g - G_PE], in_=outt.rearrange("p h w -> p (h w)"))
        yield


def _gen_ag_groups(nc, pools, x_d, xv_d, xo_d, zf_d, outv_d, groups):
    """Generator: ACT computes per-tap products into slots of [128,8,625]
    bf16 tiles; DVE folds each tile with a batched binary tree (2500/1250/
    625-wide adds, all 2x) into a chain sum, then combines the 6 chain
    sums + leftover tap.  Yields after each ACT batch and each DVE fold
    so the driver can pace ACT (evacs must not queue behind mults)."""
    xp, zp, op = pools["xa"], pools["zfa"], pools["oa"]
    tp, t1p, t2p, sp = pools["ta"], pools["t1"], pools["t2"], pools["sa"]
    for g in groups:
        split = g == G - 1
        zf_sb = zp.tile([128, NT], mybir.dt.float32, name=f"zfa{g}", tag="zfa")
        if split:
            # the DVE-affine taps need 4B-aligned windows: stride-32 rows
            # plus an odd-shifted copy (host-prepared, contiguous DMA;
            # the last xv/xo slot belongs to the split group)
            x_e = xp.tile([128, HX, 32], mybir.dt.bfloat16, name=f"xage{g}", tag="xag")
            x_o = xp.tile([128, HX, 32], mybir.dt.bfloat16, name=f"xago{g}", tag="xago")
            nc.sync.dma_start(out=x_e, in_=xv_d[G_DVE].rearrange("p (h w) -> p h w", h=HX))
            nc.sync.dma_start(out=x_o, in_=xo_d[G_DVE].rearrange("p (h w) -> p h w", h=HX))

            def win(t):
                u, v = divmod(t, WZ)
                if v % 2 == 0:
                    return x_e[:, u : u + HO, v : v + WO]
                return x_o[:, u : u + HO, v - 1 : v - 1 + WO]
        else:
            x_sb = xp.tile([128, HX, WX], mybir.dt.bfloat16, name=f"xag{g}", tag="xag")
            nc.sync.dma_start(out=x_sb, in_=x_d[g].rearrange("p (h w) -> p h w", h=HX))

            def win(t):
                u, v = divmod(t, WZ)
                return x_sb[:, u : u + HO, v : v + WO]

        nc.sync.dma_start(out=zf_sb, in_=zf_d[g - G_PE])

        n_act = SPLIT_AG_TAPS if split else NT
        n_chain = n_act // 8          # full 8-tap chains on the ACT path
        subs = sp.tile([128, 7, NO], mybir.dt.bfloat16, name=f"sub{g}", tag="sub")
        for ci in range(n_chain):
            big = tp.tile([128, 8, NO], mybir.dt.bfloat16, name=f"big{g}_{ci}", tag="big")
            for k in range(8):
                t = ci * 8 + k
                nc.scalar.activation(
                    big[:, k], win(t),
                    mybir.ActivationFunctionType.Copy,
                    bias=0.0, scale=zf_sb[:, t : t + 1],
                )
            yield
            T1 = t1p.tile([128, 4, NO], mybir.dt.bfloat16, name=f"T1_{g}_{ci}", tag="T1")
            nc.vector.tensor_add(T1, big[:, 0:4], big[:, 4:8])
            T2 = t2p.tile([128, 2, NO], mybir.dt.bfloat16, name=f"T2_{g}_{ci}", tag="T2")
            nc.vector.tensor_add(T2, T1[:, 0:2], T1[:, 2:4])
            nc.vector.tensor_add(subs[:, ci], T2[:, 0], T2[:, 1])
            yield
        if not split:
            # leftover tap 48 straight into subs slot 6
            nc.scalar.activation(
                subs[:, 6], win(NT - 1),
                mybir.ActivationFunctionType.Copy,
                bias=0.0, scale=zf_sb[:, NT - 1 : NT],
            )
            # combine 7 sums: C1[3] = subs[0:3]+subs[3:6]; fold + subs[6]
            C1 = t1p.tile([128, 3, NO], mybir.dt.bfloat16, name=f"C1_{g}", tag="T1")
            nc.vector.tensor_add(C1, subs[:, 0:3], subs[:, 3:6])
            a1 = t2p.tile([128, NO], mybir.dt.bfloat16, name=f"a1_{g}", tag="T2")
            nc.vector.tensor_add(a1, C1[:, 0], C1[:, 1])
            a2 = t2p.tile([128, NO], mybir.dt.bfloat16, name=f"a2_{g}", tag="T2")
            nc.vector.tensor_add(a2, a1, C1[:, 2])
            outt = op.tile([128, NO], mybir.dt.bfloat16, name=f"oag{g}", tag="oag")
            nc.vector.tensor_add(outt, a2, subs[:, 6])
            nc.sync.dma_start(out=outv_d[g - G_PE], in_=outt)
            yield
        else:
            # split group: taps [SPLIT_AG_TAPS, 49) via DVE affine MACs,
            # merged with the 4 ACT chain sums at the end.
            acc = t2p.tile([128, HO, WO], mybir.dt.float32, name=f"acs{g}", tag="acs")
            accb = t2p.tile([128, HO, WO], mybir.dt.bfloat16, name=f"acb{g}", tag="acb")
            for t in range(SPLIT_AG_TAPS, NT):
                if t == SPLIT_AG_TAPS:
                    nc.vector.tensor_scalar_mul(acc, win(t), zf_sb[:, t : t + 1])
                elif t < NT - 1:
                    nc.vector.affine_then_add(acc, win(t), acc, zf_sb[:, t : t + 1], 0.0)
                else:
                    nc.vector.affine_then_add(accb, win(t), acc, zf_sb[:, t : t + 1], 0.0)
                if t % 4 == 3:
                    yield
            yield
            # combine 4 chain sums + DVE partial
            C1 = t1p.tile([128, 2, NO], mybir.dt.bfloat16, name=f"C1_{g}", tag="T1")
            nc.vector.tensor_add(C1, subs[:, 0:2], subs[:, 2:4])
            a1 = t2p.tile([128, NO], mybir.dt.bfloat16, name=f"a1_{g}", tag="T2")
            nc.vector.tensor_add(a1, C1[:, 0], C1[:, 1])
            outt = op.tile([128, NO], mybir.dt.bfloat16, name=f"oag{g}", tag="oag")
            nc.vector.tensor_add(outt, a1, accb.rearrange("p h w -> p (h w)"))
            nc.sync.dma_start(out=outv_d[g - G_PE], in_=outt)
            yield


def _build():
    """Build + compile the SPMD Bass program (cached per process)."""
    if "nc" in _built:
        return _built["nc"]

    nc = bacc.Bacc(
        "TRN2", target_bir_lowering=False, debug=False, num_devices=NCORES
    )
    x_d = nc.dram_tensor("x", [G, 128, NX], mybir.dt.bfloat16, kind="ExternalInput").ap()
    # aligned (stride-32 rows) + odd-shifted copies for the affine paths:
    # slots 0..G_DVE-1 = solo DVE groups, slot G_DVE = split AG group
    xv_d = nc.dram_tensor(
        "xv", [G_DVE + 1, 128, HX * 32], mybir.dt.bfloat16, kind="ExternalInput"
    ).ap()
    xo_d = nc.dram_tensor(
        "xo", [G_DVE + 1, 128, HX * 32], mybir.dt.bfloat16, kind="ExternalInput"
    ).ap()
    zd_d = nc.dram_tensor(
        "zd", [G_PE, 128, NT, 128], mybir.dt.bfloat16, kind="ExternalInput"
    ).ap()
    zf_d = nc.dram_tensor(
        "zf", [G - G_PE, 128, NT], mybir.dt.float32, kind="ExternalInput"
    ).ap()
    out_d = nc.dram_tensor(
        "out", [G_PE, 128, NO], mybir.dt.float32, kind="ExternalOutput"
    ).ap()
    outv_d = nc.dram_tensor(
        "outv", [G - G_PE, 128, NO], mybir.dt.bfloat16, kind="ExternalOutput"
    ).ap()

    with tile.TileContext(nc) as tc:
        with (
            tc.tile_pool(name="xp", bufs=3) as xp,
            tc.tile_pool(name="zp", bufs=3) as zp,
            tc.tile_pool(name="op", bufs=3) as op,
            tc.tile_pool(name="xv", bufs=2) as xv,
            tc.tile_pool(name="zf", bufs=2) as zf,
            tc.tile_pool(name="ov", bufs=2) as ov,
            tc.tile_pool(name="ovb", bufs=2) as ovb,
            tc.tile_pool(name="xa", bufs=2) as xa,
            tc.tile_pool(name="zfa", bufs=2) as zfa,
            tc.tile_pool(name="oa", bufs=2) as oa,
            tc.tile_pool(name="ta", bufs=3) as ta,
            tc.tile_pool(name="t1", bufs=2) as t1,
            tc.tile_pool(name="t2", bufs=3) as t2,
            tc.tile_pool(name="sa", bufs=2) as sa,
            tc.tile_pool(name="psA", bufs=PSUM_BUFS, space="PSUM") as psA,
            tc.tile_pool(name="psB", bufs=PSUM_BUFS, space="PSUM") as psB,
        ):
            pools = dict(xp=xp, zp=zp, op=op, xv=xv, zf=zf, ov=ov, ovb=ovb,
                         xa=xa, zfa=zfa, oa=oa, ta=ta, t1=t1, t2=t2, sa=sa,
                         psA=psA, psB=psB)
            # PE p-state warmup: ~6us of dummy matmuls bridge the initial
            # DMA wait and ramp the clock (0.65 -> 2.4 GHz needs 3us of
            # continuous execution), so tap-0 runs at full speed.  GpSimd
            # zeroes the weight tile: its queue is free and runs the
            # memset during the framework preamble (~6us), well before
            # DVE's queue would get to it.
            warm_w = op.tile([128, 64], mybir.dt.bfloat16, name="warmw", tag="warm")
            nc.gpsimd.memset(warm_w, 0)
            # share the pB ring (no spare PSUM banks for a dedicated tile)
            warm_ps = psB.tile([64, 64], mybir.dt.float32, name="warmps", tag="pB")
            for i in range(64):
                nc.tensor.matmul(
                    warm_ps[:, :], warm_w, warm_w[:, 0:64],
                    start=True, stop=True, skip_group_check=True,
                )

            dve_gen = _gen_dve_groups(
                nc, pools, xv_d, xo_d, zf_d, outv_d, range(G_PE, G_PE + G_DVE)
            )
            # split group first so its DVE-affine taps land early in DVE's
            # queue instead of trailing 12us past PE's finish
            ag_gen = _gen_ag_groups(
                nc, pools, x_d, xv_d, xo_d, zf_d, outv_d,
                [G - 1] + list(range(G_PE + G_DVE, G - 1)),
            )
            from collections import deque

            pending_evacs = deque()
            for g in range(G_PE):
                # evac FIRST so it sits ahead of this round's AG mults in
                # ACT's queue (PSUM frees on time; no PE starvation)
                if len(pending_evacs) >= EVAC_DEFER:
                    _emit_pe_evac(nc, pools, out_d, *pending_evacs.popleft())
                if g == 0:
                    # round 0 interleaves DMA dispatch (~0.65us serialized
                    # on the sync queue each) so DVE and ACT get their
                    # first inputs ~12us in instead of ~18:
                    #   gating loads -> first DVE + AG group loads -> rest
                    g0 = _emit_g0_gating_dmas(nc, pools, x_d, zd_d)
                    next(dve_gen, None)   # emits xv/xo/zf + first taps
                    next(ag_gen, None)    # emits split-group loads + chunk
                    pA, pB = _emit_g0_rest(nc, pools, zd_d, *g0)
                else:
                    pA, pB = _emit_pe_matmuls(nc, pools, x_d, zd_d, g)
                pending_evacs.append((g, pA, pB))
                # pace: ~2 AG chunks + ~2-3 DVE affine batches per round
                # (2.5 chunks/round overloads ACT: 16us/round vs PE's 13.8,
                # backlog delays evacs and stalls PE on PSUM)
                for _ in range(3 if g == 0 else 4):
                    next(ag_gen, None)
                for _ in range((3 if g % 2 else 2) - (1 if g == 0 else 0)):
                    next(dve_gen, None)
            while pending_evacs:
                _emit_pe_evac(nc, pools, out_d, *pending_evacs.popleft())
            for _ in ag_gen:
                pass
            for _ in dve_gen:
                pass

    nc.compile()
    _built["nc"] = nc
    return nc


def _host_prep(z_f: np.ndarray, x_f: np.ndarray):
    """Shard + reformat inputs for the 8 cores."""
    x = np.ascontiguousarray(x_f, dtype=np.float32).reshape(B, C, NX)
    z = np.ascontiguousarray(z_f, dtype=np.float32).reshape(B, C, NT)
    in_maps = []
    p_idx = np.arange(128)
    aff_groups = list(range(G_PE, G_PE + G_DVE)) + [G - 1]
    for k in range(NCORES):
        xs = x[k * BPC : (k + 1) * BPC].reshape(G, 128, NX).astype(BF16)
        zs = z[k * BPC : (k + 1) * BPC].reshape(G, 128, NT)
        zd = np.zeros((G_PE, 128, NT, 128), dtype=BF16)
        # zd[g, p, t, p] = z[g*128+p, t]
        zd[:, p_idx, :, p_idx] = zs[:G_PE].astype(BF16).transpose(1, 0, 2)
        zfl = np.ascontiguousarray(zs[G_PE:])  # fp32 for DVE/AG scalar slots
        # aligned + odd-shifted copies for the affine paths (stride-32 rows
        # so every window read is 4B-aligned; built host-side so the DMA is
        # contiguous per partition)
        xg = xs[aff_groups].reshape(len(aff_groups), 128, HX, WX)
        xv = np.zeros((len(aff_groups), 128, HX, 32), dtype=BF16)
        xo = np.zeros((len(aff_groups), 128, HX, 32), dtype=BF16)
        xv[:, :, :, :WX] = xg
        xo[:, :, :, : WX - 1] = xg[:, :, :, 1:]
        in_maps.append({
            "x": xs, "zd": zd, "zf": zfl,
            "xv": xv.reshape(len(aff_groups), 128, HX * 32),
            "xo": xo.reshape(len(aff_groups), 128, HX * 32),
        })
    return in_maps


def _run(z_f, x_f, trace=False, **spmd_kwargs):
    nc = _build()
    in_maps = _host_prep(z_f, x_f)
    if trace:
        _ensure_ntff_hook()
        # local profiling only — skip the artifact share upload
        import concourse.bass_utils as _bu

        _bu.upload_artifacts = lambda tmpdir: tmpdir
    res = None
    for attempt in range(3):
        try:
            res = run_bass_kernel_spmd(
                nc, in_maps, core_ids=list(range(NCORES)), trace=trace, **spmd_kwargs
            )
            break
        except Exception:
            # the device occasionally reports a transient unrecoverable
            # state on the first touch after another process exits;
            # re-running recovers it
            if attempt == 2:
                raise
            import time

            time.sleep(5)
    full = np.empty((B, C, HO, WO), np.float32)
    fv = full.reshape(NCORES, G, 128, NO)
    for k, r in enumerate(res.results):
        fv[k, :G_PE] = np.asarray(r["out"], dtype=np.float32)
        fv[k, G_PE:] = np.asarray(r["outv"]).astype(np.float32)
    return full, res


def kernel(z_f: np.ndarray, x_f: np.ndarray) -> np.ndarray:
    full, _ = _run(z_f, x_f, trace=False)
    return full


# revision 30
# speedup vs baseline: 1.0082x; 1.0082x over previous
"""Depthwise cross-correlation (DepthwiseRPN) on 8 TRN2 NeuronCores.

Reference op:
  z_f: [B=128, C=256, 7, 7]   per-(b,c) kernels
  x_f: [B=128, C=256, 31, 31] search windows
  out: [B=128, C=256, 25, 25] valid cross-correlation per (b,c)

Sharding: pure data-parallel over B (16 batches per core).

Depthwise conv has no operand shared across a matmul grid, so TensorE
can only do ~128 useful MACs/cycle (diagonal weights; rhs-ingest
bound).  To beat the PE-only floor (~420 us/core) the per-core work is
split across three parallel pipelines by channel group (128 ch each):

  - PE groups (22): per-tap diagonal matmul, 49 taps accumulate in
    PSUM: psum[c,:] += diag(z[:,u,v]) @ x[:, shifted-window AP].
  - DVE groups (4): fused MAC via the AFFINE_THEN_ADD custom DVE op:
    acc = x_win*z_tap + acc  (bf16 reads, fp32 accumulator).
  - ACT+DVE groups (6): ScalarE mult (activation Copy with per-partition
    scale) into slots of [128,8,625] bf16 tiles; DVE folds each 8-slot
    tile with a batched binary tree (2500/1250/625-wide adds, all 2x
    mode), then combines the chain sums.

Scheduling (the v1->v2 delta, worth ~20 us):
  - PSUM pools are 4 deep and each round's evacuation is emitted BEFORE
    that round's matmuls, so the evac sits AHEAD of the round's AG mults
    in ACT's queue.  v1 accumulated ~0.6 us/round of ACT backlog which
    stalled PE 11 us near the end (PSUM starvation).
  - Group 0's x/zd DMAs are split fine (rows 0:7/7:14/14:20/20:31 and
    zd taps 0:1/1:8/8:49) and its matmuls run pA-before-pB so tap-0
    only waits on x rows 0:20 + zd tap 0 (~5 us startup vs 11.5).
  - Non-PE groups return bf16 (host upcasts); final DVE op writes the
    bf16 out tile directly.
Measured: ~334 us (v1) -> target ~310 us, max rel err ~6e-3.
"""

import numpy as np
import ml_dtypes

import concourse.bass as bass
import concourse.mybir as mybir
import concourse.tile as tile
from concourse import bacc
from concourse.bass_utils import run_bass_kernel_spmd

B, C = 128, 256
HX, WX = 31, 31
HZ, WZ = 7, 7
HO, WO = HX - HZ + 1, WX - WZ + 1  # 25, 25
NCORES = 8
BPC = B // NCORES         # batches per core = 16
Q = BPC * C               # (b,c) channels per core = 4096
G = Q // 128              # groups of 128 channels = 32
NX = HX * WX              # 961
NO = HO * WO              # 625
NT = HZ * WZ              # 49 taps
ROWS_A = 20               # psum chunk A rows (20*25=500 <= 512)
ROWS_B = HO - ROWS_A      # 5 rows (125)

# channel-group split across engines
G_PE = 21                 # TensorE diag-matmul groups
G_DVE = 4                 # DVE fused-MAC (AFFINE_THEN_ADD) groups
G_AG = G - G_PE - G_DVE   # 7: ACT-mult + DVE tree-add groups (last one split
                          # between ACT taps and DVE-affine taps)
SPLIT_AG_TAPS = 32        # last group: taps [0,32) on ACT path, rest on DVE

PSUM_BUFS = 4             # psA 4*2000B + psB 4*500B = 10KB <= 16KB
EVAC_DEFER = 3            # evac(g-EVAC_DEFER) emitted at round-g start

BF16 = ml_dtypes.bfloat16

_built = {}


def _ensure_ntff_hook():
    """Install the axon NTFF profiling hook if the container's antenv stub
    lacks it (needed only for trace=True local profiling runs)."""
    import contextlib
    import ctypes
    import sys
    import types

    try:
        from antenv.axon_hooks import get_axon_ntff_profile_hook  # noqa: F401

        return True
    except ImportError:
        pass
    so_path = "/opt/axon/libaxon_pjrt.so"
    try:
        lib = ctypes.CDLL(so_path)
    except OSError:
        return False
    if not hasattr(lib, "axon_start_nrt_profile"):
        return False
    lib.axon_start_nrt_profile.argtypes = [
        ctypes.POINTER(ctypes.c_int64),
        ctypes.c_size_t,
    ]
    lib.axon_start_nrt_profile.restype = ctypes.c_int64
    lib.axon_stop_nrt_profile.argtypes = [ctypes.c_char_p]
    lib.axon_stop_nrt_profile.restype = ctypes.c_int64

    @contextlib.contextmanager
    def _hook(output_dir, device_ids):
        import jax

        jax.devices()
        if device_ids:
            ids = (ctypes.c_int64 * len(device_ids))(*device_ids)
            rc = lib.axon_start_nrt_profile(ids, len(device_ids))
        else:
            rc = lib.axon_start_nrt_profile(None, 0)
        if rc != 0:
            raise RuntimeError(f"axon_start_nrt_profile rc={rc}")
        try:
            yield
        finally:
            n = lib.axon_stop_nrt_profile(str(output_dir).encode())
            print(f"profile: {n} file(s) written to {output_dir}", file=sys.stderr)

    state = {"hook": _hook}
    mod = types.ModuleType("antenv.axon_hooks")
    mod.get_axon_ntff_profile_hook = lambda: state["hook"]
    mod.set_axon_ntff_profile_hook = lambda h: state.update(hook=h)
    import antenv

    sys.modules["antenv.axon_hooks"] = mod
    antenv.axon_hooks = mod
    return True


def _emit_g0_gating_dmas(nc, pools, x_d, zd_d):
    """Group 0's tap-0..19 inputs only — the minimum to get PE rolling.
    The matmul dep tracker waits on whole tiles, so the zd load is split
    into three tiles sized to land just before their taps are reached."""
    xp, zp = pools["xp"], pools["zp"]
    x_sb = xp.tile([128, HX, WX], mybir.dt.bfloat16, name="xpe0", tag="xpe")
    zd_a = zp.tile([128, 8, 128], mybir.dt.bfloat16, name="zda", tag="zda", bufs=1)
    zd_b = zp.tile([128, 20, 128], mybir.dt.bfloat16, name="zdb", tag="zdb", bufs=1)
    x_src = x_d[0].rearrange("p (h w) -> p h w", h=HX)
    nc.sync.dma_start(out=zd_a, in_=zd_d[0][:, 0:8])
    nc.sync.dma_start(out=x_sb[:, 0:26], in_=x_src[:, 0:26])
    nc.sync.dma_start(out=zd_b, in_=zd_d[0][:, 8:28])
    return x_sb, x_src, zd_a, zd_b


def _emit_g0_rest(nc, pools, zd_d, x_sb, x_src, zd_a, zd_b):
    """The remaining g0 loads + the pA-then-pB matmul sweeps."""
    zp, psA, psB = pools["zp"], pools["psA"], pools["psB"]
    zd_c = zp.tile([128, NT - 28, 128], mybir.dt.bfloat16, name="zdc", tag="zdc", bufs=1)
    nc.sync.dma_start(out=zd_c, in_=zd_d[0][:, 28:])
    nc.sync.dma_start(out=x_sb[:, 26:HX], in_=x_src[:, 26:HX])

    def lhs(t):
        if t < 8:
            return zd_a[:, t, :]
        if t < 28:
            return zd_b[:, t - 8, :]
        return zd_c[:, t - 28, :]

    pA = psA.tile([128, ROWS_A * WO], mybir.dt.float32, name="pA0", tag="pA")
    pB = psB.tile([128, ROWS_B * WO], mybir.dt.float32, name="pB0", tag="pB")
    for t in range(NT):
        u, v = divmod(t, WZ)
        nc.tensor.matmul(
            pA[:, :], lhs(t), x_sb[:, u : u + ROWS_A, v : v + WO],
            start=(t == 0), stop=(t == NT - 1),
        )
    for t in range(NT):
        u, v = divmod(t, WZ)
        nc.tensor.matmul(
            pB[:, :], lhs(t),
            x_sb[:, ROWS_A + u : ROWS_A + u + ROWS_B, v : v + WO],
            start=(t == 0), stop=(t == NT - 1),
        )
    return pA, pB


def _emit_pe_matmuls(nc, pools, x_d, zd_d, g):
    """Emit one PE group's DMAs + 49 tap matmul pairs; return psum tiles
    for a deferred evacuation."""
    xp, zp, psA, psB = pools["xp"], pools["zp"], pools["psA"], pools["psB"]
    x_sb = xp.tile([128, HX, WX], mybir.dt.bfloat16, name=f"xpe{g}", tag="xpe")
    zd_sb = zp.tile([128, NT, 128], mybir.dt.bfloat16, name=f"zd{g}", tag="zd")
    x_src = x_d[g].rearrange("p (h w) -> p h w", h=HX)
    nc.sync.dma_start(out=x_sb, in_=x_src)
    nc.sync.dma_start(out=zd_sb, in_=zd_d[g])

    pA = psA.tile([128, ROWS_A * WO], mybir.dt.float32, name=f"pA{g}", tag="pA")
    pB = psB.tile([128, ROWS_B * WO], mybir.dt.float32, name=f"pB{g}", tag="pB")
    if g == G_PE - 1:
        # last group: pA finishes ~6us before pB so its evac overlaps
        # the pB sweep, trimming the tail
        for t in range(NT):
            u, v = divmod(t, WZ)
            nc.tensor.matmul(
                pA[:, :], zd_sb[:, t, :], x_sb[:, u : u + ROWS_A, v : v + WO],
                start=(t == 0), stop=(t == NT - 1),
            )
        for t in range(NT):
            u, v = divmod(t, WZ)
            nc.tensor.matmul(
                pB[:, :], zd_sb[:, t, :],
                x_sb[:, ROWS_A + u : ROWS_A + u + ROWS_B, v : v + WO],
                start=(t == 0), stop=(t == NT - 1),
            )
    else:
        for t in range(NT):
            u, v = divmod(t, WZ)
            lhsT = zd_sb[:, t, :]
            nc.tensor.matmul(
                pA[:, :], lhsT, x_sb[:, u : u + ROWS_A, v : v + WO],
                start=(t == 0), stop=(t == NT - 1),
            )
            nc.tensor.matmul(
                pB[:, :], lhsT, x_sb[:, ROWS_A + u : ROWS_A + u + ROWS_B, v : v + WO],
                start=(t == 0), stop=(t == NT - 1),
            )
    return pA, pB


def _emit_pe_evac(nc, pools, out_d, g, pA, pB, tail=False):
    op = pools["op"]
    out_sb = op.tile([128, NO], mybir.dt.float32, name=f"ope{g}", tag="ope")
    # ScalarE is closest to PSUM; keep DVE free for its MAC pipeline
    nc.scalar.copy(out=out_sb[:, : ROWS_A * WO], in_=pA[:, :])
    nc.scalar.copy(out=out_sb[:, ROWS_A * WO :], in_=pB[:, :])
    if tail:
        # the sync queue is backlogged at kernel end; dispatching from the
        # ACT queue (idle by then) starts the final writebacks immediately
        nc.scalar.dma_start(out=out_d[g], in_=out_sb)
    else:
        nc.sync.dma_start(out=out_d[g], in_=out_sb)


def _gen_dve_groups(nc, pools, xv_d, xo_d, zf_d, outv_d, groups, taps_per_yield=4):
    """Generator: DVE fused-MAC pipeline over `groups`, yielding every few
    taps so the driver can interleave c-group adds into DVE's stream."""
    xp, zp, op, ob = pools["xv"], pools["zf"], pools["ov"], pools["ovb"]
    for i, g in enumerate(groups):
        # stride-32 rows + an odd-shifted copy keep every window read
        # 4B-aligned (bf16 reads at odd element offsets run ~2x slower).
        # Both layouts are prepared host-side so the DMA is 128 contiguous
        # descriptors (a strided on-the-fly copy is ~4000 tiny descriptors
        # and serializes the sync queue for multiple us per load).
        x_e = xp.tile([128, HX, 32], mybir.dt.bfloat16, name=f"xdve{g}", tag="xdve")
        x_o = xp.tile([128, HX, 32], mybir.dt.bfloat16, name=f"xdvo{g}", tag="xdvo")
        zf_sb = zp.tile([128, NT], mybir.dt.float32, name=f"zfv{g}", tag="zfv")
        nc.sync.dma_start(out=x_e, in_=xv_d[i].rearrange("p (h w) -> p h w", h=HX))
        nc.sync.dma_start(out=x_o, in_=xo_d[i].rearrange("p (h w) -> p h w", h=HX))
        nc.sync.dma_start(out=zf_sb, in_=zf_d[g - G_PE])

        acc = op.tile([128, HO, WO], mybir.dt.float32, name=f"accv{g}", tag="accv")
        outt = ob.tile([128, HO, WO], mybir.dt.bfloat16, name=f"ovb{g}", tag="ovb")
        for t in range(NT):
            u, v = divmod(t, WZ)
            if v % 2 == 0:
                win = x_e[:, u : u + HO, v : v + WO]
            else:
                win = x_o[:, u : u + HO, v - 1 : v - 1 + WO]
            if t == 0:
                # seed on DVE (2x_2p tensor-scalar); ACT stays free for
                # evacs + AG mults
                nc.vector.tensor_scalar_mul(acc, win, zf_sb[:, 0:1])
            elif t < NT - 1:
                nc.vector.affine_then_add(acc, win, acc, zf_sb[:, t : t + 1], 0.0)
            else:
                # last tap writes the bf16 out tile directly
                nc.vector.affine_then_add(outt, win, acc, zf_sb[:, t : t + 1], 0.0)
            if (t + 1) % taps_per_yield == 0:
                yield
        nc.sync.dma_start(out=outv_d[g - G_PE], in_=outt.rearrange("p h w -> p (h w)"))
        yield


def _gen_ag_groups(nc, pools, x_d, xv_d, xo_d, zf_d, outv_d, groups):
    """Generator: ACT computes per-tap products into slots of [128,8,625]
    bf16 tiles; DVE folds each tile with a batched binary tree (2500/1250/
    625-wide adds, all 2x) into a chain sum, then combines the 6 chain
    sums + leftover tap.  Yields after each ACT batch and each DVE fold
    so the driver can pace ACT (evacs must not queue behind mults)."""
    xp, zp, op = pools["xa"], pools["zfa"], pools["oa"]
    tp, t1p, t2p, sp = pools["ta"], pools["t1"], pools["t2"], pools["sa"]
    for g in groups:
        split = g == G - 1
        zf_sb = zp.tile([128, NT], mybir.dt.float32, name=f"zfa{g}", tag="zfa")
        if split:
            # the DVE-affine taps need 4B-aligned windows: stride-32 rows
            # plus an odd-shifted copy (host-prepared, contiguous DMA;
            # the last xv/xo slot belongs to the split group)
            x_e = xp.tile([128, HX, 32], mybir.dt.bfloat16, name=f"xage{g}", tag="xag")
            x_o = xp.tile([128, HX, 32], mybir.dt.bfloat16, name=f"xago{g}", tag="xago")
            nc.sync.dma_start(out=x_e, in_=xv_d[G_DVE].rearrange("p (h w) -> p h w", h=HX))
            nc.sync.dma_start(out=x_o, in_=xo_d[G_DVE].rearrange("p (h w) -> p h w", h=HX))

            def win(t):
                u, v = divmod(t, WZ)
                if v % 2 == 0:
                    return x_e[:, u : u + HO, v : v + WO]
                return x_o[:, u : u + HO, v - 1 : v - 1 + WO]
        else:
            x_sb = xp.tile([128, HX, WX], mybir.dt.bfloat16, name=f"xag{g}", tag="xag")
            nc.sync.dma_start(out=x_sb, in_=x_d[g].rearrange("p (h w) -> p h w", h=HX))

            def win(t):
                u, v = divmod(t, WZ)
                return x_sb[:, u : u + HO, v : v + WO]

        nc.sync.dma_start(out=zf_sb, in_=zf_d[g - G_PE])

        n_act = SPLIT_AG_TAPS if split else NT
        n_chain = n_act // 8          # full 8-tap chains on the ACT path
        subs = sp.tile([128, 7, NO], mybir.dt.bfloat16, name=f"sub{g}", tag="sub")
        for ci in range(n_chain):
            big = tp.tile([128, 8, NO], mybir.dt.bfloat16, name=f"big{g}_{ci}", tag="big")
            for k in range(8):
                t = ci * 8 + k
                nc.scalar.activation(
                    big[:, k], win(t),
                    mybir.ActivationFunctionType.Copy,
                    bias=0.0, scale=zf_sb[:, t : t + 1],
                )
            yield
            T1 = t1p.tile([128, 4, NO], mybir.dt.bfloat16, name=f"T1_{g}_{ci}", tag="T1")
            nc.vector.tensor_add(T1, big[:, 0:4], big[:, 4:8])
            T2 = t2p.tile([128, 2, NO], mybir.dt.bfloat16, name=f"T2_{g}_{ci}", tag="T2")
            nc.vector.tensor_add(T2, T1[:, 0:2], T1[:, 2:4])
            nc.vector.tensor_add(subs[:, ci], T2[:, 0], T2[:, 1])
            yield
        if not split:
            # leftover tap 48 straight into subs slot 6
            nc.scalar.activation(
                subs[:, 6], win(NT - 1),
                mybir.ActivationFunctionType.Copy,
                bias=0.0, scale=zf_sb[:, NT - 1 : NT],
            )
            # combine 7 sums: C1[3] = subs[0:3]+subs[3:6]; fold + subs[6]
            C1 = t1p.tile([128, 3, NO], mybir.dt.bfloat16, name=f"C1_{g}", tag="T1")
            nc.vector.tensor_add(C1, subs[:, 0:3], subs[:, 3:6])
            a1 = t2p.tile([128, NO], mybir.dt.bfloat16, name=f"a1_{g}", tag="T2")
            nc.vector.tensor_add(a1, C1[:, 0], C1[:, 1])
            a2 = t2p.tile([128, NO], mybir.dt.bfloat16, name=f"a2_{g}", tag="T2")
            nc.vector.tensor_add(a2, a1, C1[:, 2])
            outt = op.tile([128, NO], mybir.dt.bfloat16, name=f"oag{g}", tag="oag")
            nc.vector.tensor_add(outt, a2, subs[:, 6])
            nc.sync.dma_start(out=outv_d[g - G_PE], in_=outt)
            yield
        else:
            # split group: taps [SPLIT_AG_TAPS, 49) via DVE affine MACs,
            # merged with the 4 ACT chain sums at the end.
            acc = t2p.tile([128, HO, WO], mybir.dt.float32, name=f"acs{g}", tag="acs")
            accb = t2p.tile([128, HO, WO], mybir.dt.bfloat16, name=f"acb{g}", tag="acb")
            for t in range(SPLIT_AG_TAPS, NT):
                if t == SPLIT_AG_TAPS:
                    nc.vector.tensor_scalar_mul(acc, win(t), zf_sb[:, t : t + 1])
                elif t < NT - 1:
                    nc.vector.affine_then_add(acc, win(t), acc, zf_sb[:, t : t + 1], 0.0)
                else:
                    nc.vector.affine_then_add(accb, win(t), acc, zf_sb[:, t : t + 1], 0.0)
                if t % 4 == 3:
                    yield
            yield
            # combine 4 chain sums + DVE partial
            C1 = t1p.tile([128, 2, NO], mybir.dt.bfloat16, name=f"C1_{g}", tag="T1")
            nc.vector.tensor_add(C1, subs[:, 0:2], subs[:, 2:4])
            a1 = t2p.tile([128, NO], mybir.dt.bfloat16, name=f"a1_{g}", tag="T2")
            nc.vector.tensor_add(a1, C1[:, 0], C1[:, 1])
            outt = op.tile([128, NO], mybir.dt.bfloat16, name=f"oag{g}", tag="oag")
            nc.vector.tensor_add(outt, a1, accb.rearrange("p h w -> p (h w)"))
            nc.sync.dma_start(out=outv_d[g - G_PE], in_=outt)
            yield


def _build():
    """Build + compile the SPMD Bass program (cached per process)."""
    if "nc" in _built:
        return _built["nc"]

    nc = bacc.Bacc(
        "TRN2", target_bir_lowering=False, debug=False, num_devices=NCORES
    )
    x_d = nc.dram_tensor("x", [G, 128, NX], mybir.dt.bfloat16, kind="ExternalInput").ap()
    # aligned (stride-32 rows) + odd-shifted copies for the affine paths:
    # slots 0..G_DVE-1 = solo DVE groups, slot G_DVE = split AG group
    xv_d = nc.dram_tensor(
        "xv", [G_DVE + 1, 128, HX * 32], mybir.dt.bfloat16, kind="ExternalInput"
    ).ap()
    xo_d = nc.dram_tensor(
        "xo", [G_DVE + 1, 128, HX * 32], mybir.dt.bfloat16, kind="ExternalInput"
    ).ap()
    zd_d = nc.dram_tensor(
        "zd", [G_PE, 128, NT, 128], mybir.dt.bfloat16, kind="ExternalInput"
    ).ap()
    zf_d = nc.dram_tensor(
        "zf", [G - G_PE, 128, NT], mybir.dt.float32, kind="ExternalInput"
    ).ap()
    out_d = nc.dram_tensor(
        "out", [G_PE, 128, NO], mybir.dt.float32, kind="ExternalOutput"
    ).ap()
    outv_d = nc.dram_tensor(
        "outv", [G - G_PE, 128, NO], mybir.dt.bfloat16, kind="ExternalOutput"
    ).ap()

    with tile.TileContext(nc) as tc:
        with (
            tc.tile_pool(name="xp", bufs=3) as xp,
            tc.tile_pool(name="zp", bufs=3) as zp,
            tc.tile_pool(name="op", bufs=3) as op,
            tc.tile_pool(name="xv", bufs=2) as xv,
            tc.tile_pool(name="zf", bufs=2) as zf,
            tc.tile_pool(name="ov", bufs=2) as ov,
            tc.tile_pool(name="ovb", bufs=2) as ovb,
            tc.tile_pool(name="xa", bufs=2) as xa,
            tc.tile_pool(name="zfa", bufs=2) as zfa,
            tc.tile_pool(name="oa", bufs=2) as oa,
            tc.tile_pool(name="ta", bufs=3) as ta,
            tc.tile_pool(name="t1", bufs=2) as t1,
            tc.tile_pool(name="t2", bufs=3) as t2,
            tc.tile_pool(name="sa", bufs=2) as sa,
            tc.tile_pool(name="psA", bufs=PSUM_BUFS, space="PSUM") as psA,
            tc.tile_pool(name="psB", bufs=PSUM_BUFS, space="PSUM") as psB,
        ):
            pools = dict(xp=xp, zp=zp, op=op, xv=xv, zf=zf, ov=ov, ovb=ovb,
                         xa=xa, zfa=zfa, oa=oa, ta=ta, t1=t1, t2=t2, sa=sa,
                         psA=psA, psB=psB)
            # PE p-state warmup: ~6us of dummy matmuls bridge the initial
            # DMA wait and ramp the clock (0.65 -> 2.4 GHz needs 3us of
            # continuous execution), so tap-0 runs at full speed.  GpSimd
            # zeroes the weight tile: its queue is free and runs the
            # memset during the framework preamble (~6us), well before
            # DVE's queue would get to it.
            warm_w = op.tile([128, 64], mybir.dt.bfloat16, name="warmw", tag="warm")
            nc.gpsimd.memset(warm_w, 0)
            # share the pB ring (no spare PSUM banks for a dedicated tile)
            warm_ps = psB.tile([64, 64], mybir.dt.float32, name="warmps", tag="pB")
            for i in range(64):
                nc.tensor.matmul(
                    warm_ps[:, :], warm_w, warm_w[:, 0:64],
                    start=True, stop=True, skip_group_check=True,
                )

            dve_gen = _gen_dve_groups(
                nc, pools, xv_d, xo_d, zf_d, outv_d, range(G_PE, G_PE + G_DVE)
            )
            # split group first so its DVE-affine taps land early in DVE's
            # queue instead of trailing 12us past PE's finish
            ag_gen = _gen_ag_groups(
                nc, pools, x_d, xv_d, xo_d, zf_d, outv_d,
                [G - 1] + list(range(G_PE + G_DVE, G - 1)),
            )
            from collections import deque

            pending_evacs = deque()
            for g in range(G_PE):
                # evac FIRST so it sits ahead of this round's AG mults in
                # ACT's queue (PSUM frees on time; no PE starvation)
                if len(pending_evacs) >= EVAC_DEFER:
                    _emit_pe_evac(nc, pools, out_d, *pending_evacs.popleft())
                if g == 0:
                    # round 0 interleaves DMA dispatch (~0.65us serialized
                    # on the sync queue each) so DVE and ACT get their
                    # first inputs ~12us in instead of ~18:
                    #   gating loads -> first DVE + AG group loads -> rest
                    g0 = _emit_g0_gating_dmas(nc, pools, x_d, zd_d)
                    next(dve_gen, None)   # emits xv/xo/zf + first taps
                    next(ag_gen, None)    # emits split-group loads + chunk
                    pA, pB = _emit_g0_rest(nc, pools, zd_d, *g0)
                else:
                    pA, pB = _emit_pe_matmuls(nc, pools, x_d, zd_d, g)
                pending_evacs.append((g, pA, pB))
                # pace: ~2 AG chunks + ~2-3 DVE affine batches per round
                # (2.5 chunks/round overloads ACT: 16us/round vs PE's 13.8,
                # backlog delays evacs and stalls PE on PSUM)
                for _ in range(3 if g == 0 else 4):
                    next(ag_gen, None)
                for _ in range((3 if g % 2 else 2) - (1 if g == 0 else 0)):
                    next(dve_gen, None)
            while pending_evacs:
                _emit_pe_evac(nc, pools, out_d, *pending_evacs.popleft(), tail=True)
            for _ in ag_gen:
                pass
            for _ in dve_gen:
                pass

    nc.compile()
    _built["nc"] = nc
    return nc


def _host_prep(z_f: np.ndarray, x_f: np.ndarray):
    """Shard + reformat inputs for the 8 cores."""
    x = np.ascontiguousarray(x_f, dtype=np.float32).reshape(B, C, NX)
    z = np.ascontiguousarray(z_f, dtype=np.float32).reshape(B, C, NT)
    in_maps = []
    p_idx = np.arange(128)
    aff_groups = list(range(G_PE, G_PE + G_DVE)) + [G - 1]
    for k in range(NCORES):
        xs = x[k * BPC : (k + 1) * BPC].reshape(G, 128, NX).astype(BF16)
        zs = z[k * BPC : (k + 1) * BPC].reshape(G, 128, NT)
        zd = np.zeros((G_PE, 128, NT, 128), dtype=BF16)
        # zd[g, p, t, p] = z[g*128+p, t]
        zd[:, p_idx, :, p_idx] = zs[:G_PE].astype(BF16).transpose(1, 0, 2)
        zfl = np.ascontiguousarray(zs[G_PE:])  # fp32 for DVE/AG scalar slots
        # aligned + odd-shifted copies for the affine paths (stride-32 rows
        # so every window read is 4B-aligned; built host-side so the DMA is
        # contiguous per partition)
        xg = xs[aff_groups].reshape(len(aff_groups), 128, HX, WX)
        xv = np.zeros((len(aff_groups), 128, HX, 32), dtype=BF16)
        xo = np.zeros((len(aff_groups), 128, HX, 32), dtype=BF16)
        xv[:, :, :, :WX] = xg
        xo[:, :, :, : WX - 1] = xg[:, :, :, 1:]
        in_maps.append({
            "x": xs, "zd": zd, "zf": zfl,
            "xv": xv.reshape(len(aff_groups), 128, HX * 32),
            "xo": xo.reshape(len(aff_groups), 128, HX * 32),
        })
    return in_maps


def _run(z_f, x_f, trace=False, **spmd_kwargs):
    nc = _build()
    in_maps = _host_prep(z_f, x_f)
    if trace:
        _ensure_ntff_hook()
        # local profiling only — skip the artifact share upload
        import concourse.bass_utils as _bu

        _bu.upload_artifacts = lambda tmpdir: tmpdir
    res = None
    for attempt in range(3):
        try:
            res = run_bass_kernel_spmd(
                nc, in_maps, core_ids=list(range(NCORES)), trace=trace, **spmd_kwargs
            )
            break
        except Exception:
            # the device occasionally reports a transient unrecoverable
            # state on the first touch after another process exits;
            # re-running recovers it
            if attempt == 2:
                raise
            import time

            time.sleep(5)
    full = np.empty((B, C, HO, WO), np.float32)
    fv = full.reshape(NCORES, G, 128, NO)
    for k, r in enumerate(res.results):
        fv[k, :G_PE] = np.asarray(r["out"], dtype=np.float32)
        fv[k, G_PE:] = np.asarray(r["outv"]).astype(np.float32)
    return full, res


def kernel(z_f: np.ndarray, x_f: np.ndarray) -> np.ndarray:
    full, _ = _run(z_f, x_f, trace=False)
    return full


# revision 35
# speedup vs baseline: 1.0103x; 1.0020x over previous
"""Depthwise cross-correlation (DepthwiseRPN) on 8 TRN2 NeuronCores.

Reference op:
  z_f: [B=128, C=256, 7, 7]   per-(b,c) kernels
  x_f: [B=128, C=256, 31, 31] search windows
  out: [B=128, C=256, 25, 25] valid cross-correlation per (b,c)

Sharding: pure data-parallel over B (16 batches per core).

Depthwise conv has no operand shared across a matmul grid, so TensorE
can only do ~128 useful MACs/cycle (diagonal weights; rhs-ingest
bound).  To beat the PE-only floor (~420 us/core) the per-core work is
split across three parallel pipelines by channel group (128 ch each):

  - PE groups (22): per-tap diagonal matmul, 49 taps accumulate in
    PSUM: psum[c,:] += diag(z[:,u,v]) @ x[:, shifted-window AP].
  - DVE groups (4): fused MAC via the AFFINE_THEN_ADD custom DVE op:
    acc = x_win*z_tap + acc  (bf16 reads, fp32 accumulator).
  - ACT+DVE groups (6): ScalarE mult (activation Copy with per-partition
    scale) into slots of [128,8,625] bf16 tiles; DVE folds each 8-slot
    tile with a batched binary tree (2500/1250/625-wide adds, all 2x
    mode), then combines the chain sums.

Scheduling (the v1->v2 delta, worth ~20 us):
  - PSUM pools are 4 deep and each round's evacuation is emitted BEFORE
    that round's matmuls, so the evac sits AHEAD of the round's AG mults
    in ACT's queue.  v1 accumulated ~0.6 us/round of ACT backlog which
    stalled PE 11 us near the end (PSUM starvation).
  - Group 0's x/zd DMAs are split fine (rows 0:7/7:14/14:20/20:31 and
    zd taps 0:1/1:8/8:49) and its matmuls run pA-before-pB so tap-0
    only waits on x rows 0:20 + zd tap 0 (~5 us startup vs 11.5).
  - Non-PE groups return bf16 (host upcasts); final DVE op writes the
    bf16 out tile directly.
Measured: ~334 us (v1) -> target ~310 us, max rel err ~6e-3.
"""

import numpy as np
import ml_dtypes

import concourse.bass as bass
import concourse.mybir as mybir
import concourse.tile as tile
from concourse import bacc
from concourse.bass_utils import run_bass_kernel_spmd

B, C = 128, 256
HX, WX = 31, 31
HZ, WZ = 7, 7
HO, WO = HX - HZ + 1, WX - WZ + 1  # 25, 25
NCORES = 8
BPC = B // NCORES         # batches per core = 16
Q = BPC * C               # (b,c) channels per core = 4096
G = Q // 128              # groups of 128 channels = 32
NX = HX * WX              # 961
NO = HO * WO              # 625
NT = HZ * WZ              # 49 taps
ROWS_A = 20               # psum chunk A rows (20*25=500 <= 512)
ROWS_B = HO - ROWS_A      # 5 rows (125)

# channel-group split across engines
G_PE = 21                 # TensorE diag-matmul groups
G_DVE = 4                 # DVE fused-MAC (AFFINE_THEN_ADD) groups
G_AG = G - G_PE - G_DVE   # 7: ACT-mult + DVE tree-add groups (last one split
                          # between ACT taps and DVE-affine taps)
SPLIT_AG_TAPS = 40        # last group: taps [0,40) on ACT path, rest on DVE

PSUM_BUFS = 4             # psA 4*2000B + psB 4*500B = 10KB <= 16KB
EVAC_DEFER = 3            # evac(g-EVAC_DEFER) emitted at round-g start

BF16 = ml_dtypes.bfloat16

_built = {}


def _ensure_ntff_hook():
    """Install the axon NTFF profiling hook if the container's antenv stub
    lacks it (needed only for trace=True local profiling runs)."""
    import contextlib
    import ctypes
    import sys
    import types

    try:
        from antenv.axon_hooks import get_axon_ntff_profile_hook  # noqa: F401

        return True
    except ImportError:
        pass
    so_path = "/opt/axon/libaxon_pjrt.so"
    try:
        lib = ctypes.CDLL(so_path)
    except OSError:
        return False
    if not hasattr(lib, "axon_start_nrt_profile"):
        return False
    lib.axon_start_nrt_profile.argtypes = [
        ctypes.POINTER(ctypes.c_int64),
        ctypes.c_size_t,
    ]
    lib.axon_start_nrt_profile.restype = ctypes.c_int64
    lib.axon_stop_nrt_profile.argtypes = [ctypes.c_char_p]
    lib.axon_stop_nrt_profile.restype = ctypes.c_int64

    @contextlib.contextmanager
    def _hook(output_dir, device_ids):
        import jax

        jax.devices()
        if device_ids:
            ids = (ctypes.c_int64 * len(device_ids))(*device_ids)
            rc = lib.axon_start_nrt_profile(ids, len(device_ids))
        else:
            rc = lib.axon_start_nrt_profile(None, 0)
        if rc != 0:
            raise RuntimeError(f"axon_start_nrt_profile rc={rc}")
        try:
            yield
        finally:
            n = lib.axon_stop_nrt_profile(str(output_dir).encode())
            print(f"profile: {n} file(s) written to {output_dir}", file=sys.stderr)

    state = {"hook": _hook}
    mod = types.ModuleType("antenv.axon_hooks")
    mod.get_axon_ntff_profile_hook = lambda: state["hook"]
    mod.set_axon_ntff_profile_hook = lambda h: state.update(hook=h)
    import antenv

    sys.modules["antenv.axon_hooks"] = mod
    antenv.axon_hooks = mod
    return True


def _emit_g0_gating_dmas(nc, pools, x_d, zd_d):
    """Group 0's tap-0..19 inputs only — the minimum to get PE rolling.
    The matmul dep tracker waits on whole tiles, so the zd load is split
    into three tiles sized to land just before their taps are reached."""
    xp, zp = pools["xp"], pools["zp"]
    x_sb = xp.tile([128, HX, WX], mybir.dt.bfloat16, name="xpe0", tag="xpe")
    zd_a = zp.tile([128, 8, 128], mybir.dt.bfloat16, name="zda", tag="zda", bufs=1)
    zd_b = zp.tile([128, 20, 128], mybir.dt.bfloat16, name="zdb", tag="zdb", bufs=1)
    x_src = x_d[0].rearrange("p (h w) -> p h w", h=HX)
    nc.sync.dma_start(out=zd_a, in_=zd_d[0][:, 0:8])
    nc.sync.dma_start(out=x_sb[:, 0:26], in_=x_src[:, 0:26])
    return x_sb, x_src, zd_a, zd_b


def _emit_g0_zdb(nc, zd_d, zd_b):
    nc.sync.dma_start(out=zd_b, in_=zd_d[0][:, 8:28])


def _emit_g0_rest(nc, pools, zd_d, x_sb, x_src, zd_a, zd_b):
    """The remaining g0 loads + the pA-then-pB matmul sweeps."""
    zp, psA, psB = pools["zp"], pools["psA"], pools["psB"]
    zd_c = zp.tile([128, NT - 28, 128], mybir.dt.bfloat16, name="zdc", tag="zdc", bufs=1)
    nc.sync.dma_start(out=zd_c, in_=zd_d[0][:, 28:])
    nc.sync.dma_start(out=x_sb[:, 26:HX], in_=x_src[:, 26:HX])

    def lhs(t):
        if t < 8:
            return zd_a[:, t, :]
        if t < 28:
            return zd_b[:, t - 8, :]
        return zd_c[:, t - 28, :]

    pA = psA.tile([128, ROWS_A * WO], mybir.dt.float32, name="pA0", tag="pA")
    pB = psB.tile([128, ROWS_B * WO], mybir.dt.float32, name="pB0", tag="pB")
    for t in range(NT):
        u, v = divmod(t, WZ)
        nc.tensor.matmul(
            pA[:, :], lhs(t), x_sb[:, u : u + ROWS_A, v : v + WO],
            start=(t == 0), stop=(t == NT - 1),
        )
    for t in range(NT):
        u, v = divmod(t, WZ)
        nc.tensor.matmul(
            pB[:, :], lhs(t),
            x_sb[:, ROWS_A + u : ROWS_A + u + ROWS_B, v : v + WO],
            start=(t == 0), stop=(t == NT - 1),
        )
    return pA, pB


def _emit_pe_matmuls(nc, pools, x_d, zd_d, g):
    """Emit one PE group's DMAs + 49 tap matmul pairs; return psum tiles
    for a deferred evacuation."""
    xp, zp, psA, psB = pools["xp"], pools["zp"], pools["psA"], pools["psB"]
    x_sb = xp.tile([128, HX, WX], mybir.dt.bfloat16, name=f"xpe{g}", tag="xpe")
    zd_sb = zp.tile([128, NT, 128], mybir.dt.bfloat16, name=f"zd{g}", tag="zd")
    x_src = x_d[g].rearrange("p (h w) -> p h w", h=HX)
    nc.sync.dma_start(out=x_sb, in_=x_src)
    nc.sync.dma_start(out=zd_sb, in_=zd_d[g])

    pA = psA.tile([128, ROWS_A * WO], mybir.dt.float32, name=f"pA{g}", tag="pA")
    pB = psB.tile([128, ROWS_B * WO], mybir.dt.float32, name=f"pB{g}", tag="pB")
    if g == G_PE - 1:
        # last group: pA finishes ~6us before pB so its evac overlaps
        # the pB sweep, trimming the tail
        for t in range(NT):
            u, v = divmod(t, WZ)
            nc.tensor.matmul(
                pA[:, :], zd_sb[:, t, :], x_sb[:, u : u + ROWS_A, v : v + WO],
                start=(t == 0), stop=(t == NT - 1),
            )
        for t in range(NT):
            u, v = divmod(t, WZ)
            nc.tensor.matmul(
                pB[:, :], zd_sb[:, t, :],
                x_sb[:, ROWS_A + u : ROWS_A + u + ROWS_B, v : v + WO],
                start=(t == 0), stop=(t == NT - 1),
            )
    else:
        for t in range(NT):
            u, v = divmod(t, WZ)
            lhsT = zd_sb[:, t, :]
            nc.tensor.matmul(
                pA[:, :], lhsT, x_sb[:, u : u + ROWS_A, v : v + WO],
                start=(t == 0), stop=(t == NT - 1),
            )
            nc.tensor.matmul(
                pB[:, :], lhsT, x_sb[:, ROWS_A + u : ROWS_A + u + ROWS_B, v : v + WO],
                start=(t == 0), stop=(t == NT - 1),
            )
    return pA, pB


def _emit_pe_evac(nc, pools, out_d, g, pA, pB, tail=False):
    op = pools["op"]
    out_sb = op.tile([128, NO], mybir.dt.float32, name=f"ope{g}", tag="ope")
    # ScalarE is closest to PSUM; keep DVE free for its MAC pipeline
    nc.scalar.copy(out=out_sb[:, : ROWS_A * WO], in_=pA[:, :])
    nc.scalar.copy(out=out_sb[:, ROWS_A * WO :], in_=pB[:, :])
    if tail:
        # the sync queue is backlogged at kernel end; dispatching from the
        # ACT queue (idle by then) starts the final writebacks immediately
        nc.scalar.dma_start(out=out_d[g], in_=out_sb)
    else:
        nc.sync.dma_start(out=out_d[g], in_=out_sb)


def _gen_dve_groups(nc, pools, xv_d, xo_d, zf_d, outv_d, groups, taps_per_yield=4):
    """Generator: DVE fused-MAC pipeline over `groups`, yielding every few
    taps so the driver can interleave c-group adds into DVE's stream."""
    xp, zp, op, ob = pools["xv"], pools["zf"], pools["ov"], pools["ovb"]
    for i, g in enumerate(groups):
        # stride-32 rows + an odd-shifted copy keep every window read
        # 4B-aligned (bf16 reads at odd element offsets run ~2x slower).
        # Both layouts are prepared host-side so the DMA is 128 contiguous
        # descriptors (a strided on-the-fly copy is ~4000 tiny descriptors
        # and serializes the sync queue for multiple us per load).
        x_e = xp.tile([128, HX, 32], mybir.dt.bfloat16, name=f"xdve{g}", tag="xdve")
        x_o = xp.tile([128, HX, 32], mybir.dt.bfloat16, name=f"xdvo{g}", tag="xdvo")
        zf_sb = zp.tile([128, NT], mybir.dt.float32, name=f"zfv{g}", tag="zfv")
        nc.sync.dma_start(out=x_e, in_=xv_d[i].rearrange("p (h w) -> p h w", h=HX))
        nc.sync.dma_start(out=x_o, in_=xo_d[i].rearrange("p (h w) -> p h w", h=HX))
        nc.sync.dma_start(out=zf_sb, in_=zf_d[g - G_PE])

        acc = op.tile([128, HO, WO], mybir.dt.float32, name=f"accv{g}", tag="accv")
        outt = ob.tile([128, HO, WO], mybir.dt.bfloat16, name=f"ovb{g}", tag="ovb")
        for t in range(NT):
            u, v = divmod(t, WZ)
            if v % 2 == 0:
                win = x_e[:, u : u + HO, v : v + WO]
            else:
                win = x_o[:, u : u + HO, v - 1 : v - 1 + WO]
            if t == 0:
                # seed on DVE (2x_2p tensor-scalar); ACT stays free for
                # evacs + AG mults
                nc.vector.tensor_scalar_mul(acc, win, zf_sb[:, 0:1])
            elif t < NT - 1:
                nc.vector.affine_then_add(acc, win, acc, zf_sb[:, t : t + 1], 0.0)
            else:
                # last tap writes the bf16 out tile directly
                nc.vector.affine_then_add(outt, win, acc, zf_sb[:, t : t + 1], 0.0)
            if (t + 1) % taps_per_yield == 0:
                yield
        nc.sync.dma_start(out=outv_d[g - G_PE], in_=outt.rearrange("p h w -> p (h w)"))
        yield


def _gen_ag_groups(nc, pools, x_d, xv_d, xo_d, zf_d, outv_d, groups):
    """Generator: ACT computes per-tap products into slots of [128,8,625]
    bf16 tiles; DVE folds each tile with a batched binary tree (2500/1250/
    625-wide adds, all 2x) into a chain sum, then combines the 6 chain
    sums + leftover tap.  Yields after each ACT batch and each DVE fold
    so the driver can pace ACT (evacs must not queue behind mults)."""
    xp, zp, op = pools["xa"], pools["zfa"], pools["oa"]
    tp, t1p, t2p, sp = pools["ta"], pools["t1"], pools["t2"], pools["sa"]
    for g in groups:
        split = g == G - 1
        zf_sb = zp.tile([128, NT], mybir.dt.float32, name=f"zfa{g}", tag="zfa")
        if split:
            # the DVE-affine taps need 4B-aligned windows: stride-32 rows
            # plus an odd-shifted copy (host-prepared, contiguous DMA;
            # the last xv/xo slot belongs to the split group)
            x_e = xp.tile([128, HX, 32], mybir.dt.bfloat16, name=f"xage{g}", tag="xag")
            x_o = xp.tile([128, HX, 32], mybir.dt.bfloat16, name=f"xago{g}", tag="xago")
            nc.sync.dma_start(out=x_e, in_=xv_d[G_DVE].rearrange("p (h w) -> p h w", h=HX))
            nc.sync.dma_start(out=x_o, in_=xo_d[G_DVE].rearrange("p (h w) -> p h w", h=HX))

            def win(t):
                u, v = divmod(t, WZ)
                if v % 2 == 0:
                    return x_e[:, u : u + HO, v : v + WO]
                return x_o[:, u : u + HO, v - 1 : v - 1 + WO]
        else:
            x_sb = xp.tile([128, HX, WX], mybir.dt.bfloat16, name=f"xag{g}", tag="xag")
            nc.sync.dma_start(out=x_sb, in_=x_d[g].rearrange("p (h w) -> p h w", h=HX))

            def win(t):
                u, v = divmod(t, WZ)
                return x_sb[:, u : u + HO, v : v + WO]

        nc.sync.dma_start(out=zf_sb, in_=zf_d[g - G_PE])

        n_act = SPLIT_AG_TAPS if split else NT
        n_chain = n_act // 8          # full 8-tap chains on the ACT path
        subs = sp.tile([128, 7, NO], mybir.dt.bfloat16, name=f"sub{g}", tag="sub")
        for ci in range(n_chain):
            big = tp.tile([128, 8, NO], mybir.dt.bfloat16, name=f"big{g}_{ci}", tag="big")
            for k in range(8):
                t = ci * 8 + k
                nc.scalar.activation(
                    big[:, k], win(t),
                    mybir.ActivationFunctionType.Copy,
                    bias=0.0, scale=zf_sb[:, t : t + 1],
                )
            yield
            T1 = t1p.tile([128, 4, NO], mybir.dt.bfloat16, name=f"T1_{g}_{ci}", tag="T1")
            nc.vector.tensor_add(T1, big[:, 0:4], big[:, 4:8])
            T2 = t2p.tile([128, 2, NO], mybir.dt.bfloat16, name=f"T2_{g}_{ci}", tag="T2")
            nc.vector.tensor_add(T2, T1[:, 0:2], T1[:, 2:4])
            nc.vector.tensor_add(subs[:, ci], T2[:, 0], T2[:, 1])
            yield
        if not split:
            # leftover tap 48 straight into subs slot 6
            nc.scalar.activation(
                subs[:, 6], win(NT - 1),
                mybir.ActivationFunctionType.Copy,
                bias=0.0, scale=zf_sb[:, NT - 1 : NT],
            )
            # combine 7 sums: C1[3] = subs[0:3]+subs[3:6]; fold + subs[6]
            C1 = t1p.tile([128, 3, NO], mybir.dt.bfloat16, name=f"C1_{g}", tag="T1")
            nc.vector.tensor_add(C1, subs[:, 0:3], subs[:, 3:6])
            a1 = t2p.tile([128, NO], mybir.dt.bfloat16, name=f"a1_{g}", tag="T2")
            nc.vector.tensor_add(a1, C1[:, 0], C1[:, 1])
            a2 = t2p.tile([128, NO], mybir.dt.bfloat16, name=f"a2_{g}", tag="T2")
            nc.vector.tensor_add(a2, a1, C1[:, 2])
            outt = op.tile([128, NO], mybir.dt.bfloat16, name=f"oag{g}", tag="oag")
            nc.vector.tensor_add(outt, a2, subs[:, 6])
            nc.sync.dma_start(out=outv_d[g - G_PE], in_=outt)
            yield
        else:
            # split group: taps [SPLIT_AG_TAPS, 49) via DVE affine MACs,
            # merged with the 4 ACT chain sums at the end.
            acc = t2p.tile([128, HO, WO], mybir.dt.float32, name=f"acs{g}", tag="acs")
            accb = t2p.tile([128, HO, WO], mybir.dt.bfloat16, name=f"acb{g}", tag="acb")
            for t in range(SPLIT_AG_TAPS, NT):
                if t == SPLIT_AG_TAPS:
                    nc.vector.tensor_scalar_mul(acc, win(t), zf_sb[:, t : t + 1])
                elif t < NT - 1:
                    nc.vector.affine_then_add(acc, win(t), acc, zf_sb[:, t : t + 1], 0.0)
                else:
                    nc.vector.affine_then_add(accb, win(t), acc, zf_sb[:, t : t + 1], 0.0)
                if t % 4 == 3:
                    yield
            yield
            # combine n_chain chain sums + DVE partial (pairwise tree)
            vals = [subs[:, ci] for ci in range(n_chain)]
            vals.append(accb.rearrange("p h w -> p (h w)"))
            li = 0
            while len(vals) > 2:
                nxt = []
                for i in range(0, len(vals) - 1, 2):
                    s = t2p.tile([128, NO], mybir.dt.bfloat16,
                                 name=f"cm{g}_{li}_{i}", tag="cmb", bufs=6)
                    nc.vector.tensor_add(s, vals[i], vals[i + 1])
                    nxt.append(s)
                if len(vals) % 2:
                    nxt.append(vals[-1])
                vals = nxt
                li += 1
            outt = op.tile([128, NO], mybir.dt.bfloat16, name=f"oag{g}", tag="oag")
            nc.vector.tensor_add(outt, vals[0], vals[1])
            nc.sync.dma_start(out=outv_d[g - G_PE], in_=outt)
            yield


def _build():
    """Build + compile the SPMD Bass program (cached per process)."""
    if "nc" in _built:
        return _built["nc"]

    nc = bacc.Bacc(
        "TRN2", target_bir_lowering=False, debug=False, num_devices=NCORES
    )
    x_d = nc.dram_tensor("x", [G, 128, NX], mybir.dt.bfloat16, kind="ExternalInput").ap()
    # aligned (stride-32 rows) + odd-shifted copies for the affine paths:
    # slots 0..G_DVE-1 = solo DVE groups, slot G_DVE = split AG group
    xv_d = nc.dram_tensor(
        "xv", [G_DVE + 1, 128, HX * 32], mybir.dt.bfloat16, kind="ExternalInput"
    ).ap()
    xo_d = nc.dram_tensor(
        "xo", [G_DVE + 1, 128, HX * 32], mybir.dt.bfloat16, kind="ExternalInput"
    ).ap()
    zd_d = nc.dram_tensor(
        "zd", [G_PE, 128, NT, 128], mybir.dt.bfloat16, kind="ExternalInput"
    ).ap()
    zf_d = nc.dram_tensor(
        "zf", [G - G_PE, 128, NT], mybir.dt.float32, kind="ExternalInput"
    ).ap()
    out_d = nc.dram_tensor(
        "out", [G_PE, 128, NO], mybir.dt.float32, kind="ExternalOutput"
    ).ap()
    outv_d = nc.dram_tensor(
        "outv", [G - G_PE, 128, NO], mybir.dt.bfloat16, kind="ExternalOutput"
    ).ap()

    with tile.TileContext(nc) as tc:
        with (
            tc.tile_pool(name="xp", bufs=3) as xp,
            tc.tile_pool(name="zp", bufs=3) as zp,
            tc.tile_pool(name="op", bufs=3) as op,
            tc.tile_pool(name="xv", bufs=2) as xv,
            tc.tile_pool(name="zf", bufs=2) as zf,
            tc.tile_pool(name="ov", bufs=2) as ov,
            tc.tile_pool(name="ovb", bufs=2) as ovb,
            tc.tile_pool(name="xa", bufs=2) as xa,
            tc.tile_pool(name="zfa", bufs=2) as zfa,
            tc.tile_pool(name="oa", bufs=2) as oa,
            tc.tile_pool(name="ta", bufs=3) as ta,
            tc.tile_pool(name="t1", bufs=2) as t1,
            tc.tile_pool(name="t2", bufs=3) as t2,
            tc.tile_pool(name="sa", bufs=2) as sa,
            tc.tile_pool(name="psA", bufs=PSUM_BUFS, space="PSUM") as psA,
            tc.tile_pool(name="psB", bufs=PSUM_BUFS, space="PSUM") as psB,
        ):
            pools = dict(xp=xp, zp=zp, op=op, xv=xv, zf=zf, ov=ov, ovb=ovb,
                         xa=xa, zfa=zfa, oa=oa, ta=ta, t1=t1, t2=t2, sa=sa,
                         psA=psA, psB=psB)
            # PE p-state warmup: ~6us of dummy matmuls bridge the initial
            # DMA wait and ramp the clock (0.65 -> 2.4 GHz needs 3us of
            # continuous execution), so tap-0 runs at full speed.  GpSimd
            # zeroes the weight tile: its queue is free and runs the
            # memset during the framework preamble (~6us), well before
            # DVE's queue would get to it.
            warm_w = op.tile([128, 64], mybir.dt.bfloat16, name="warmw", tag="warm")
            nc.gpsimd.memset(warm_w, 0)
            # share the pB ring (no spare PSUM banks for a dedicated tile)
            warm_ps = psB.tile([64, 64], mybir.dt.float32, name="warmps", tag="pB")
            for i in range(64):
                nc.tensor.matmul(
                    warm_ps[:, :], warm_w, warm_w[:, 0:64],
                    start=True, stop=True, skip_group_check=True,
                )

            dve_gen = _gen_dve_groups(
                nc, pools, xv_d, xo_d, zf_d, outv_d, range(G_PE, G_PE + G_DVE)
            )
            # split group first so its DVE-affine taps land early in DVE's
            # queue instead of trailing 12us past PE's finish
            ag_gen = _gen_ag_groups(
                nc, pools, x_d, xv_d, xo_d, zf_d, outv_d,
                [G - 1] + list(range(G_PE + G_DVE, G - 1)),
            )
            from collections import deque

            pending_evacs = deque()
            for g in range(G_PE):
                # evac FIRST so it sits ahead of this round's AG mults in
                # ACT's queue (PSUM frees on time; no PE starvation)
                if len(pending_evacs) >= EVAC_DEFER:
                    _emit_pe_evac(nc, pools, out_d, *pending_evacs.popleft())
                if g == 0:
                    # round 0 interleaves DMA dispatch (~0.65us serialized
                    # on the sync queue each) so DVE and ACT get their
                    # first inputs early:
                    #   tap-0:7 loads -> DVE group loads -> zd taps 8:28
                    #   -> AG group loads -> rest
                    g0 = _emit_g0_gating_dmas(nc, pools, x_d, zd_d)
                    next(dve_gen, None)   # emits xv/xo/zf + first taps
                    _emit_g0_zdb(nc, zd_d, g0[3])
                    next(ag_gen, None)    # emits split-group loads + chunk
                    pA, pB = _emit_g0_rest(nc, pools, zd_d, *g0)
                else:
                    pA, pB = _emit_pe_matmuls(nc, pools, x_d, zd_d, g)
                pending_evacs.append((g, pA, pB))
                # pace: ~2 AG chunks + ~2-3 DVE affine batches per round
                # (2.5 chunks/round overloads ACT: 16us/round vs PE's 13.8,
                # backlog delays evacs and stalls PE on PSUM)
                for _ in range(3 if g == 0 else 4):
                    next(ag_gen, None)
                for _ in range((3 if g % 2 else 2) - (1 if g == 0 else 0)):
                    next(dve_gen, None)
            while pending_evacs:
                _emit_pe_evac(nc, pools, out_d, *pending_evacs.popleft(), tail=True)
            for _ in ag_gen:
                pass
            for _ in dve_gen:
                pass

    nc.compile()
    _built["nc"] = nc
    return nc


def _host_prep(z_f: np.ndarray, x_f: np.ndarray):
    """Shard + reformat inputs for the 8 cores."""
    x = np.ascontiguousarray(x_f, dtype=np.float32).reshape(B, C, NX)
    z = np.ascontiguousarray(z_f, dtype=np.float32).reshape(B, C, NT)
    in_maps = []
    p_idx = np.arange(128)
    aff_groups = list(range(G_PE, G_PE + G_DVE)) + [G - 1]
    for k in range(NCORES):
        xs = x[k * BPC : (k + 1) * BPC].reshape(G, 128, NX).astype(BF16)
        zs = z[k * BPC : (k + 1) * BPC].reshape(G, 128, NT)
        zd = np.zeros((G_PE, 128, NT, 128), dtype=BF16)
        # zd[g, p, t, p] = z[g*128+p, t]
        zd[:, p_idx, :, p_idx] = zs[:G_PE].astype(BF16).transpose(1, 0, 2)
        zfl = np.ascontiguousarray(zs[G_PE:])  # fp32 for DVE/AG scalar slots
        # aligned + odd-shifted copies for the affine paths (stride-32 rows
        # so every window read is 4B-aligned; built host-side so the DMA is
        # contiguous per partition)
        xg = xs[aff_groups].reshape(len(aff_groups), 128, HX, WX)
        xv = np.zeros((len(aff_groups), 128, HX, 32), dtype=BF16)
        xo = np.zeros((len(aff_groups), 128, HX, 32), dtype=BF16)
        xv[:, :, :, :WX] = xg
        xo[:, :, :, : WX - 1] = xg[:, :, :, 1:]
        in_maps.append({
            "x": xs, "zd": zd, "zf": zfl,
            "xv": xv.reshape(len(aff_groups), 128, HX * 32),
            "xo": xo.reshape(len(aff_groups), 128, HX * 32),
        })
    return in_maps


def _run(z_f, x_f, trace=False, **spmd_kwargs):
    nc = _build()
    in_maps = _host_prep(z_f, x_f)
    if trace:
        _ensure_ntff_hook()
        # local profiling only — skip the artifact share upload
        import concourse.bass_utils as _bu

        _bu.upload_artifacts = lambda tmpdir: tmpdir
    res = None
    for attempt in range(3):
        try:
            res = run_bass_kernel_spmd(
                nc, in_maps, core_ids=list(range(NCORES)), trace=trace, **spmd_kwargs
            )
            break
        except Exception:
            # the device occasionally reports a transient unrecoverable
            # state on the first touch after another process exits;
            # re-running recovers it
            if attempt == 2:
                raise
            import time

            time.sleep(5)
    full = np.empty((B, C, HO, WO), np.float32)
    fv = full.reshape(NCORES, G, 128, NO)
    for k, r in enumerate(res.results):
        fv[k, :G_PE] = np.asarray(r["out"], dtype=np.float32)
        fv[k, G_PE:] = np.asarray(r["outv"]).astype(np.float32)
    return full, res


def kernel(z_f: np.ndarray, x_f: np.ndarray) -> np.ndarray:
    full, _ = _run(z_f, x_f, trace=False)
    return full
